# revision 23
# baseline (speedup 1.0000x reference)
"""MoChA stable chunkwise attention (window w=16) on 8 Trainium2 NeuronCores.

The reference's stabilizing moving-max cancels algebraically:
    P[t] = exp(logits[t]);  S[u] = sum_{v=u-15..u} P[v]
    R[u] = emit[u]/S[u];    out[t] = P[t] * sum_{k=0..15} R[t+k]
Both width-16 window sums run on the TensorEngine as banded matmuls in a
transposed layout: partition p = t mod 128, free column f = 8*(t//128) + row.
With that ordering the "previous block" of any column is exactly 8 columns
to the left, so the cross-block window wrap is two full-width matmuls against
shifted views of the same SBUF buffer (an 8-column zero pad supplies the
sequence-edge padding) — no masked-copy corner operands.

The four banded weight matrices are generated on-device (memset +
affine_select on the otherwise idle GpSimd engine) so no ring bandwidth is
spent on them. Logits halves load on the two HW rings, emit (packed two
partitions per dram row for 4KB lines) on the software-DGE ring. Separate
PSUM tiles per pipeline stage keep dependencies range-precise; the second
half's divide/window/store path runs in 8/256/248-column slices so its
output DMAs start as early as possible. Output returns fp16, upcast on host.

Self-contained: only numpy + concourse (on PYTHONPATH) required.
"""

import numpy as np

import concourse.bass as bass
import concourse.tile as tile
import concourse.mybir as mybir
from concourse import bacc
from concourse.bass_utils import run_bass_kernel_spmd

F32 = mybir.dt.float32
F16 = mybir.dt.float16
ACTF = mybir.ActivationFunctionType
ALU = mybir.AluOpType

B, T = 64, 16384
NCORES = 8
RPC = B // NCORES        # 8 rows/core
NPART = 128
NBG = T // NPART         # 128 blocks of 128 t's per row
NF = RPC * NBG           # 1024 free columns
W = 16                   # window
PAD = RPC                # one block-shift = 8 columns

H0 = slice(0, 512)
H1 = slice(512, 1024)


def _perm(a):
    """[RPC, T] -> device layout [128, NF]: f = 8*(t//128) + row."""
    return np.ascontiguousarray(
        a.reshape(RPC, NBG, NPART).transpose(2, 1, 0).reshape(NPART, NF)
    )


def unperm_out(o):
    """[128, NF] device layout -> [RPC, T]."""
    return np.ascontiguousarray(
        o.reshape(NPART, NBG, RPC).transpose(2, 1, 0).reshape(RPC, T)
    )


def _gen_consts(nc, kb):
    """Band weights, built in-place on GpSimd: band0[k,m]=1 iff 0<=m-k<=15;
    corner[k,m]=1 iff k-m in [113,127]; banda/cornera are the transposes."""
    g = nc.gpsimd
    # only is_ge lowers in walrus codegen; every band test is phrased >= 0.
    # S-weights first — the first matmul needs band0 earliest.
    g.memset(kb[:, 0:256], 1.0)
    # band0: keep (m-k >= 0) and (15-m+k >= 0)
    g.affine_select(kb[:, 0:128], kb[:, 0:128], [[1, 128]], ALU.is_ge, 0.0,
                    base=0, channel_multiplier=-1)
    g.affine_select(kb[:, 0:128], kb[:, 0:128], [[-1, 128]], ALU.is_ge, 0.0,
                    base=W - 1, channel_multiplier=1)
    # corner: keep (k-m-113 >= 0)
    g.affine_select(kb[:, 128:256], kb[:, 128:256], [[-1, 128]], ALU.is_ge,
                    0.0, base=-(128 - W + 1), channel_multiplier=1)


def _gen_consts_z(nc, kb):
    g = nc.gpsimd
    g.memset(kb[:, 256:512], 1.0)
    # banda: keep (k-m >= 0) and (15-k+m >= 0)
    g.affine_select(kb[:, 256:384], kb[:, 256:384], [[-1, 128]], ALU.is_ge,
                    0.0, base=0, channel_multiplier=1)
    g.affine_select(kb[:, 256:384], kb[:, 256:384], [[1, 128]], ALU.is_ge,
                    0.0, base=W - 1, channel_multiplier=-1)
    # cornera: keep (m-k-113 >= 0)
    g.affine_select(kb[:, 384:512], kb[:, 384:512], [[1, 128]], ALU.is_ge,
                    0.0, base=-(128 - W + 1), channel_multiplier=-1)


def build_nc():
    nc = bacc.Bacc("TRN2", target_bir_lowering=False, debug=False,
                   num_devices=NCORES)
    lg_t = nc.dram_tensor("lg16", [NPART, NF], F16, kind="ExternalInput")
    # emit halves, each packed two partitions per dram row (2KB lines)
    em0_t = nc.dram_tensor("em0", [NPART // 2, NF], F16,
                           kind="ExternalInput")
    em1_t = nc.dram_tensor("em1", [NPART // 2, NF], F16,
                           kind="ExternalInput")
    out_t = nc.dram_tensor("out16", [NPART, NF], F16, kind="ExternalOutput")

    with tile.TileContext(nc) as tc:
        with (
            tc.tile_pool(name="sb", bufs=1) as sb,
            tc.tile_pool(name="ps", bufs=1, space="PSUM") as ps,
        ):
            kb = sb.tile([NPART, 512], F16, tag="kb")
            lg_b = sb.tile([NPART, NF], F16, tag="lg_b")
            e_b = sb.tile([NPART, NF], F16, tag="e_b")
            p_full = sb.tile([NPART, PAD + NF], F16, tag="p_full")
            rcp_b = sb.tile([NPART, NF], F32, tag="rcp_b")
            r_full = sb.tile([NPART, NF + PAD], F16, tag="r_full")
            o_b = sb.tile([NPART, NF], F16, tag="o_b")
            s_psA = ps.tile([NPART, 512], F32, tag="sA")
            s_psB = ps.tile([NPART, 512], F32, tag="sB")
            z_psA = ps.tile([NPART, 512], F32, tag="zA")
            z_psB0 = ps.tile([NPART, 256], F32, tag="zB0")
            z_psB1 = ps.tile([NPART, 256], F32, tag="zB1")

            band0 = kb[:, 0:128]
            corner = kb[:, 128:256]
            banda = kb[:, 256:384]
            cornera = kb[:, 384:512]

            # P region of p_full is [PAD : PAD+NF]; col PAD+f holds P[f].
            pP = p_full[:, PAD:PAD + NF]

            # ---- loads: logits-A on sync, logits-B on gpsimd (no ACT-table
            # tax there), emit halves late on scalar + gpsimd; weights are
            # generated, not loaded ----
            nc.sync.dma_start(lg_b[:, H0],
                              bass.AP(lg_t, 0, [[NF, NPART], [1, 512]]))
            nc.scalar.dma_start(e_b[:, H0],
                                bass.AP(em0_t, 0, [[NF, NPART // 2],
                                                   [1, NF]]))
            nc.sync.dma_start(lg_b[:, H1],
                              bass.AP(lg_t, 512, [[NF, NPART], [1, 512]]))
            _gen_consts(nc, kb)
            nc.gpsimd.dma_start(e_b[:, H1],
                                bass.AP(em1_t, 0, [[NF, NPART // 2],
                                                   [1, NF]]))
            _gen_consts_z(nc, kb)

            # zero pads: left pad of p_full, right pad of r_full
            nc.vector.memset(p_full[:, 0:PAD], 0.0)
            nc.vector.memset(r_full[:, NF:NF + PAD], 0.0)

            # ---- P = exp(logits), fp16, halves ----
            nc.scalar.activation(pP[:, H0], lg_b[:, H0], ACTF.Exp)
            nc.scalar.activation(pP[:, H1], lg_b[:, H1], ACTF.Exp)

            # ---- S = band0.T @ P + corner.T @ P(shifted one block left) ----
            nc.tensor.matmul(s_psA[:, :], band0, pP[:, H0],
                             start=True, stop=False, skip_group_check=True)
            nc.tensor.matmul(s_psA[:, :], corner, p_full[:, 0:512],
                             start=False, stop=True, skip_group_check=True)
            nc.tensor.matmul(s_psB[:, :], band0, pP[:, H1],
                             start=True, stop=False, skip_group_check=True)
            nc.tensor.matmul(s_psB[:, :], corner, p_full[:, 512:1024],
                             start=False, stop=True, skip_group_check=True)

            # ---- R = emit / S.  Half B in 8/256/248-column slices so
            # downstream work starts as early as possible ----
            nc.vector.reciprocal_approx_fast(rcp_b[:, H0], s_psA[:, :])
            nc.vector.tensor_mul(r_full[:, H0], e_b[:, H0], rcp_b[:, H0])
            nc.vector.reciprocal_approx_fast(rcp_b[:, 512:520], s_psB[:, 0:8])
            nc.gpsimd.tensor_mul(r_full[:, 512:520], e_b[:, 512:520],
                                 rcp_b[:, 512:520])
            nc.vector.reciprocal_approx_fast(rcp_b[:, 520:776],
                                             s_psB[:, 8:264])
            nc.vector.tensor_mul(r_full[:, 520:776], e_b[:, 520:776],
                                 rcp_b[:, 520:776])
            nc.vector.reciprocal_approx_fast(rcp_b[:, 776:1024],
                                             s_psB[:, 264:512])
            nc.vector.tensor_mul(r_full[:, 776:1024], e_b[:, 776:1024],
                                 rcp_b[:, 776:1024])

            # ---- Z = banda.T @ R + cornera.T @ R(shifted one block right).
            # Half A's corner is split 504/8 (tail reads early half-B R);
            # half B runs in two 256-column pieces ----
            nc.tensor.matmul(z_psA[:, :], banda, r_full[:, H0],
                             start=True, stop=False, skip_group_check=True)
            nc.tensor.matmul(z_psA[:, 0:504], cornera, r_full[:, PAD:512],
                             start=False, stop=False, skip_group_check=True)
            nc.tensor.matmul(z_psA[:, 504:512], cornera, r_full[:, 512:520],
                             start=False, stop=True, skip_group_check=True)
            nc.tensor.matmul(z_psB0[:, :], banda, r_full[:, 512:768],
                             start=True, stop=False, skip_group_check=True)
            nc.tensor.matmul(z_psB0[:, :], cornera, r_full[:, 520:776],
                             start=False, stop=True, skip_group_check=True)
            nc.tensor.matmul(z_psB1[:, :], banda, r_full[:, 768:1024],
                             start=True, stop=False, skip_group_check=True)
            nc.tensor.matmul(z_psB1[:, :], cornera,
                             r_full[:, 776:1032],
                             start=False, stop=True, skip_group_check=True)

            # ---- out = P * Z (fp16) in 256-column pieces, DMA'd on the two
            # HW rings as soon as each piece lands ----
            nc.vector.tensor_mul(o_b[:, 0:256], pP[:, 0:256], z_psA[:, 0:256])
            nc.sync.dma_start(bass.AP(out_t, 0, [[NF, NPART], [1, 256]]),
                              o_b[:, 0:256])
            nc.vector.tensor_mul(o_b[:, 256:512], pP[:, 256:512],
                                 z_psA[:, 256:512])
            nc.scalar.dma_start(bass.AP(out_t, 256, [[NF, NPART], [1, 256]]),
                                o_b[:, 256:512])
            nc.vector.tensor_mul(o_b[:, 512:768], pP[:, 512:768],
                                 z_psB0[:, :])
            nc.sync.dma_start(bass.AP(out_t, 512, [[NF, NPART], [1, 256]]),
                              o_b[:, 512:768])
            nc.vector.tensor_mul(o_b[:, 768:1024], pP[:, 768:1024],
                                 z_psB1[:, :])
            nc.scalar.dma_start(bass.AP(out_t, 768, [[NF, NPART], [1, 256]]),
                                o_b[:, 768:1024])

    nc.compile()
    return nc


def make_in_maps(emit_probs, softmax_logits):
    lg16 = np.asarray(softmax_logits, dtype=np.float16)
    em16 = np.asarray(emit_probs, dtype=np.float16)
    maps = []
    for k in range(NCORES):
        rows = slice(k * RPC, (k + 1) * RPC)
        emp = _perm(em16[rows])
        maps.append({
            "lg16": _perm(lg16[rows]),
            "em0": np.ascontiguousarray(emp[:, 0:512]).reshape(
                NPART // 2, NF),
            "em1": np.ascontiguousarray(emp[:, 512:1024]).reshape(
                NPART // 2, NF),
        })
    return maps


_NC_CACHE = None


def _get_nc():
    global _NC_CACHE
    if _NC_CACHE is None:
        _NC_CACHE = build_nc()
    return _NC_CACHE


def run(emit_probs, softmax_logits, trace=False, **kwargs):
    nc = _get_nc()
    in_maps = make_in_maps(emit_probs, softmax_logits)
    res = run_bass_kernel_spmd(
        nc, in_maps, core_ids=list(range(NCORES)), trace=trace, **kwargs
    )
    out = np.concatenate(
        [unperm_out(res.results[k]["out16"]).astype(np.float32)
         for k in range(NCORES)],
        axis=0,
    )
    return out, res


def kernel(emit_probs, softmax_logits):
    return run(emit_probs, softmax_logits)[0]


# revision 25
# speedup vs baseline: 1.0268x; 1.0268x over previous
"""MoChA stable chunkwise attention (window w=16) on 8 Trainium2 NeuronCores.

The reference's stabilizing moving-max cancels algebraically:
    P[t] = exp(logits[t]);  S[u] = sum_{v=u-15..u} P[v]
    R[u] = emit[u]/S[u];    out[t] = P[t] * sum_{k=0..15} R[t+k]
Both width-16 window sums run on the TensorEngine as banded matmuls in a
transposed layout: partition p = t mod 128, free column f = 8*(t//128) + row.
With that ordering the "previous block" of any column is exactly 8 columns
to the left, so the cross-block window wrap is two full-width matmuls against
shifted views of the same SBUF buffer (an 8-column zero pad supplies the
sequence-edge padding) — no masked-copy corner operands.

The four banded weight matrices are generated on-device (memset +
affine_select on the otherwise idle GpSimd engine) so no ring bandwidth is
spent on them. Both logits halves stream back-to-back on the sync ring;
emit halves (packed two partitions per dram row for 2KB lines) ride the
scalar and software-DGE rings. Separate PSUM tiles per pipeline stage keep
dependencies range-precise; the second half's divide/window/store path runs
in 8/256/248-column slices so its output DMAs start as early as possible.
Output returns fp16, upcast on host.

Self-contained: only numpy + concourse (on PYTHONPATH) required.
"""

import numpy as np

import concourse.bass as bass
import concourse.tile as tile
import concourse.mybir as mybir
from concourse import bacc
from concourse.bass_utils import run_bass_kernel_spmd

F32 = mybir.dt.float32
F16 = mybir.dt.float16
ACTF = mybir.ActivationFunctionType
ALU = mybir.AluOpType

B, T = 64, 16384
NCORES = 8
RPC = B // NCORES        # 8 rows/core
NPART = 128
NBG = T // NPART         # 128 blocks of 128 t's per row
NF = RPC * NBG           # 1024 free columns
W = 16                   # window
PAD = RPC                # one block-shift = 8 columns

H0 = slice(0, 512)
H1 = slice(512, 1024)


def _perm(a):
    """[RPC, T] -> device layout [128, NF]: f = 8*(t//128) + row."""
    return np.ascontiguousarray(
        a.reshape(RPC, NBG, NPART).transpose(2, 1, 0).reshape(NPART, NF)
    )


def unperm_out(o):
    """[128, NF] device layout -> [RPC, T]."""
    return np.ascontiguousarray(
        o.reshape(NPART, NBG, RPC).transpose(2, 1, 0).reshape(RPC, T)
    )


def _gen_consts(nc, kb):
    """Band weights, built in-place on GpSimd: band0[k,m]=1 iff 0<=m-k<=15;
    corner[k,m]=1 iff k-m in [113,127]; banda/cornera are the transposes."""
    g = nc.gpsimd
    # only is_ge lowers in walrus codegen; every band test is phrased >= 0.
    # S-weights first — the first matmul needs band0 earliest.
    g.memset(kb[:, 0:256], 1.0)
    # band0: keep (m-k >= 0) and (15-m+k >= 0)
    g.affine_select(kb[:, 0:128], kb[:, 0:128], [[1, 128]], ALU.is_ge, 0.0,
                    base=0, channel_multiplier=-1)
    g.affine_select(kb[:, 0:128], kb[:, 0:128], [[-1, 128]], ALU.is_ge, 0.0,
                    base=W - 1, channel_multiplier=1)
    # corner: keep (k-m-113 >= 0)
    g.affine_select(kb[:, 128:256], kb[:, 128:256], [[-1, 128]], ALU.is_ge,
                    0.0, base=-(128 - W + 1), channel_multiplier=1)


def _gen_consts_z(nc, kb):
    g = nc.gpsimd
    g.memset(kb[:, 256:512], 1.0)
    # banda: keep (k-m >= 0) and (15-k+m >= 0)
    g.affine_select(kb[:, 256:384], kb[:, 256:384], [[-1, 128]], ALU.is_ge,
                    0.0, base=0, channel_multiplier=1)
    g.affine_select(kb[:, 256:384], kb[:, 256:384], [[1, 128]], ALU.is_ge,
                    0.0, base=W - 1, channel_multiplier=-1)
    # cornera: keep (m-k-113 >= 0)
    g.affine_select(kb[:, 384:512], kb[:, 384:512], [[1, 128]], ALU.is_ge,
                    0.0, base=-(128 - W + 1), channel_multiplier=-1)


def build_nc():
    nc = bacc.Bacc("TRN2", target_bir_lowering=False, debug=False,
                   num_devices=NCORES)
    lg_t = nc.dram_tensor("lg16", [NPART, NF], F16, kind="ExternalInput")
    # emit halves, each packed two partitions per dram row (2KB lines)
    em0_t = nc.dram_tensor("em0", [NPART // 2, NF], F16,
                           kind="ExternalInput")
    em1_t = nc.dram_tensor("em1", [NPART // 2, NF], F16,
                           kind="ExternalInput")
    out_t = nc.dram_tensor("out16", [NPART, NF], F16, kind="ExternalOutput")

    with tile.TileContext(nc) as tc:
        with (
            tc.tile_pool(name="sb", bufs=1) as sb,
            tc.tile_pool(name="ps", bufs=1, space="PSUM") as ps,
        ):
            kb = sb.tile([NPART, 512], F16, tag="kb")
            lg_b = sb.tile([NPART, NF], F16, tag="lg_b")
            e_b = sb.tile([NPART, NF], F16, tag="e_b")
            p_full = sb.tile([NPART, PAD + NF], F16, tag="p_full")
            rcp_b = sb.tile([NPART, NF], F32, tag="rcp_b")
            r_full = sb.tile([NPART, NF + PAD], F16, tag="r_full")
            o_b = sb.tile([NPART, NF], F16, tag="o_b")
            s_psA = ps.tile([NPART, 512], F32, tag="sA")
            s_psB = ps.tile([NPART, 512], F32, tag="sB")
            z_psA = ps.tile([NPART, 512], F32, tag="zA")
            z_psB0 = ps.tile([NPART, 256], F32, tag="zB0")
            z_psB1 = ps.tile([NPART, 256], F32, tag="zB1")

            band0 = kb[:, 0:128]
            corner = kb[:, 128:256]
            banda = kb[:, 256:384]
            cornera = kb[:, 384:512]

            # P region of p_full is [PAD : PAD+NF]; col PAD+f holds P[f].
            pP = p_full[:, PAD:PAD + NF]

            # ---- loads: logits-A on sync, logits-B on gpsimd (no ACT-table
            # tax there), emit halves late on scalar + gpsimd; weights are
            # generated, not loaded ----
            nc.sync.dma_start(lg_b[:, H0],
                              bass.AP(lg_t, 0, [[NF, NPART], [1, 512]]))
            nc.sync.dma_start(lg_b[:, H1],
                              bass.AP(lg_t, 512, [[NF, NPART], [1, 512]]))
            # consts generate while logits stream; emit issues are delayed
            # behind them so the logits transfers get the HBM bandwidth
            _gen_consts(nc, kb)
            nc.gpsimd.dma_start(e_b[:, H1],
                                bass.AP(em1_t, 0, [[NF, NPART // 2],
                                                   [1, NF]]))
            _gen_consts_z(nc, kb)

            # zero pads: left pad of p_full, right pad of r_full
            nc.vector.memset(p_full[:, 0:PAD], 0.0)
            nc.vector.memset(r_full[:, NF:NF + PAD], 0.0)

            # ---- P = exp(logits), fp16, halves ----
            nc.scalar.activation(pP[:, H0], lg_b[:, H0], ACTF.Exp)
            nc.scalar.dma_start(e_b[:, H0],
                                bass.AP(em0_t, 0, [[NF, NPART // 2],
                                                   [1, NF]]))
            nc.scalar.activation(pP[:, H1], lg_b[:, H1], ACTF.Exp)

            # ---- S = band0.T @ P + corner.T @ P(shifted one block left) ----
            nc.tensor.matmul(s_psA[:, :], band0, pP[:, H0],
                             start=True, stop=False, skip_group_check=True)
            nc.tensor.matmul(s_psA[:, :], corner, p_full[:, 0:512],
                             start=False, stop=True, skip_group_check=True)
            nc.tensor.matmul(s_psB[:, :], band0, pP[:, H1],
                             start=True, stop=False, skip_group_check=True)
            nc.tensor.matmul(s_psB[:, :], corner, p_full[:, 512:1024],
                             start=False, stop=True, skip_group_check=True)

            # ---- R = emit / S.  Half B in 8/256/248-column slices so
            # downstream work starts as early as possible ----
            nc.vector.reciprocal_approx_fast(rcp_b[:, H0], s_psA[:, :])
            nc.vector.tensor_mul(r_full[:, H0], e_b[:, H0], rcp_b[:, H0])
            nc.vector.reciprocal_approx_fast(rcp_b[:, 512:520], s_psB[:, 0:8])
            nc.gpsimd.tensor_mul(r_full[:, 512:520], e_b[:, 512:520],
                                 rcp_b[:, 512:520])
            nc.vector.reciprocal_approx_fast(rcp_b[:, 520:776],
                                             s_psB[:, 8:264])
            nc.vector.tensor_mul(r_full[:, 520:776], e_b[:, 520:776],
                                 rcp_b[:, 520:776])
            nc.vector.reciprocal_approx_fast(rcp_b[:, 776:1024],
                                             s_psB[:, 264:512])
            nc.vector.tensor_mul(r_full[:, 776:1024], e_b[:, 776:1024],
                                 rcp_b[:, 776:1024])

            # ---- Z = banda.T @ R + cornera.T @ R(shifted one block right).
            # Half A's corner is split 504/8 (tail reads early half-B R);
            # half B runs in two 256-column pieces ----
            nc.tensor.matmul(z_psA[:, :], banda, r_full[:, H0],
                             start=True, stop=False, skip_group_check=True)
            nc.tensor.matmul(z_psA[:, 0:504], cornera, r_full[:, PAD:512],
                             start=False, stop=False, skip_group_check=True)
            nc.tensor.matmul(z_psA[:, 504:512], cornera, r_full[:, 512:520],
                             start=False, stop=True, skip_group_check=True)
            nc.tensor.matmul(z_psB0[:, :], banda, r_full[:, 512:768],
                             start=True, stop=False, skip_group_check=True)
            nc.tensor.matmul(z_psB0[:, :], cornera, r_full[:, 520:776],
                             start=False, stop=True, skip_group_check=True)
            nc.tensor.matmul(z_psB1[:, :], banda, r_full[:, 768:1024],
                             start=True, stop=False, skip_group_check=True)
            nc.tensor.matmul(z_psB1[:, :], cornera,
                             r_full[:, 776:1032],
                             start=False, stop=True, skip_group_check=True)

            # ---- out = P * Z (fp16) in 256-column pieces, DMA'd on the two
            # HW rings as soon as each piece lands ----
            nc.vector.tensor_mul(o_b[:, 0:256], pP[:, 0:256], z_psA[:, 0:256])
            nc.sync.dma_start(bass.AP(out_t, 0, [[NF, NPART], [1, 256]]),
                              o_b[:, 0:256])
            nc.vector.tensor_mul(o_b[:, 256:512], pP[:, 256:512],
                                 z_psA[:, 256:512])
            nc.scalar.dma_start(bass.AP(out_t, 256, [[NF, NPART], [1, 256]]),
                                o_b[:, 256:512])
            nc.vector.tensor_mul(o_b[:, 512:768], pP[:, 512:768],
                                 z_psB0[:, :])
            nc.sync.dma_start(bass.AP(out_t, 512, [[NF, NPART], [1, 256]]),
                              o_b[:, 512:768])
            nc.vector.tensor_mul(o_b[:, 768:1024], pP[:, 768:1024],
                                 z_psB1[:, :])
            nc.scalar.dma_start(bass.AP(out_t, 768, [[NF, NPART], [1, 256]]),
                                o_b[:, 768:1024])

    nc.compile()
    return nc


def make_in_maps(emit_probs, softmax_logits):
    lg16 = np.asarray(softmax_logits, dtype=np.float16)
    em16 = np.asarray(emit_probs, dtype=np.float16)
    maps = []
    for k in range(NCORES):
        rows = slice(k * RPC, (k + 1) * RPC)
        emp = _perm(em16[rows])
        maps.append({
            "lg16": _perm(lg16[rows]),
            "em0": np.ascontiguousarray(emp[:, 0:512]).reshape(
                NPART // 2, NF),
            "em1": np.ascontiguousarray(emp[:, 512:1024]).reshape(
                NPART // 2, NF),
        })
    return maps


_NC_CACHE = None


def _get_nc():
    global _NC_CACHE
    if _NC_CACHE is None:
        _NC_CACHE = build_nc()
    return _NC_CACHE


def run(emit_probs, softmax_logits, trace=False, **kwargs):
    nc = _get_nc()
    in_maps = make_in_maps(emit_probs, softmax_logits)
    res = run_bass_kernel_spmd(
        nc, in_maps, core_ids=list(range(NCORES)), trace=trace, **kwargs
    )
    out = np.concatenate(
        [unperm_out(res.results[k]["out16"]).astype(np.float32)
         for k in range(NCORES)],
        axis=0,
    )
    return out, res


def kernel(emit_probs, softmax_logits):
    return run(emit_probs, softmax_logits)[0]
